# revision 16
# baseline (speedup 1.0000x reference)
"""BioZorro sparse-attention kernel for 8 Trainium2 NeuronCores.

Sharding: 8 cores = 2 batches x 4 token-quarters (384 own tokens each).
The zorro mask makes all non-fusion query rows fully masked -> uniform
softmax -> their attention output is mean(V) over all tokens; only the 16
fusion tokens do real attention (over the 1536 non-fusion keys).

v2 restructure vs baseline:
- LN mean-centering is folded into pre-centered consumer weights on the
  host (W - colmean(W)), so every LN inside the layer stack is a pure
  per-token scale y = x * rstd.
- Two exchanges per layer: EX1 ships ysum (row sums of the LN1 output,
  2KB) immediately after LN1; with the host-precomputed Mvo = Wv @ wo/N
  the uniform-attention delta du = Mvo^T ysum_all + wo_n^T vfu is ready
  while attention partials are still being computed, so FF1 over the 384
  own tokens starts ~6us into the layer and fully hides EX2 (the 64KB
  flash-softmax partial exchange). Fusion columns (16) are patched in
  post-EX2 with cheap narrow matmuls.
- One packed weight tensor per layer, streamed with a handful of wide
  DMAs; PE stream ordered so the tensor engine never idles long enough
  to lose its clock ramp.
"""
import sys
sys.path.insert(0, "/opt/trn_rl_repo")
import numpy as np
import ml_dtypes

BF = ml_dtypes.bfloat16
OWN, FUS, TOK = 384, 16, 400
D, RIN, H, DH, IFF, DEPTH = 512, 1024, 8, 64, 1365, 4
IFFP = 1408
W2T = 11
NALL = 1552
B, NR, NA = 2, 768, 768
N_CORES = 8

# packed per-layer weight layout (columns of a [128, FTOT] bf16 tile)
OKV = 0            # wkv: 4 chunks x 1024
OQ = 4096          # wq: 4 chunks x 512
OMV = 6144         # Mvo: 4 chunks x 512
OWN_ = 8192        # wo_n: 4 chunks x 512
OW1 = 10240        # w1 j-major: 11 j x (4 kc x (x 128 | g 128)) = 11 x 1024
OWOH = 21504       # woh: 8 heads x 512 (on partitions 0:64)
OW2 = 25600        # w2: 11 j x 512
FTOT = 31232

_built = {}
PHASES = []


def _mark(nc, name):
    nm = nc.get_next_instruction_name()
    PHASES.append((name, int(nm.split("-")[1])))


def build(num_devices=8, use_cc=True):
    key = (num_devices, use_cc)
    if key in _built:
        return _built[key]
    import concourse.tile as tile
    from concourse import bacc, mybir
    from concourse.masks import make_identity

    # Force Exp to resolve to natural_log_exp_and_others so the Ln/Exp
    # pairs in the LN rstd chain share one ACT table set.
    if not getattr(bacc, "_act_tables_patched", False):
        _orig_gat = bacc.get_activation_tables

        def _patched_gat(arch):
            tabs = _orig_gat(arch)
            exp_t = mybir.ActivationFunctionType.Exp
            for nm, fns in tabs.items():
                if nm != "natural_log_exp_and_others":
                    fns.discard(exp_t)
            return tabs

        bacc.get_activation_tables = _patched_gat
        bacc._act_tables_patched = True

    f32 = mybir.dt.float32
    bf16 = mybir.dt.bfloat16
    AF = mybir.ActivationFunctionType
    OP = mybir.AluOpType

    nc = bacc.Bacc("TRN2", target_bir_lowering=False, debug=False,
                   enable_asserts=True, num_devices=num_devices)

    def din(name, shape, dt=f32):
        return nc.dram_tensor(name, shape, dt, kind="ExternalInput").ap()

    x_t = din("x_t", [RIN, OWN], bf16)
    ew_t = din("emb_w", [RIN, D], bf16)
    ebias_t = din("emb_b", [D, 1])
    eg2 = din("eln2_g", [D, 1])
    eb2 = din("eln2_b", [D, 1])
    fus_t = din("fus_t", [D, FUS], bf16)
    wpack_t = din("wpack", [DEPTH, 128, FTOT], bf16)
    ppack_t = din("ppack", [128, 12288], bf16)
    pq2_t = din("pool_q2", [D, 1])
    out_u = nc.dram_tensor("out_u", [D, 1], f32, kind="ExternalOutput").ap()
    out_f = nc.dram_tensor("out_f", [1, D], f32, kind="ExternalOutput").ap()

    # pool pack layout: pkv 4x1024 | pwoh 8x512 (64 parts) | pwon 4x512 | Mvop 4x512
    POKV, POWOH, POWON, POMV = 0, 4096, 8192, 10240

    tok_chunks = [(0, 128), (128, 256), (256, 384), (384, 400)]
    rg = [[0, 1, 2, 3], [4, 5, 6, 7]]

    with tile.TileContext(nc) as tc:
        with tc.tile_pool(name="cst", bufs=1) as cst, \
             tc.tile_pool(name="wp", bufs=2) as wp, \
             tc.tile_pool(name="ac", bufs=2) as ac, \
             tc.tile_pool(name="pp", bufs=2, space="PSUM") as pp, \
             tc.tile_pool(name="dramp", bufs=2, space="DRAM") as dramp:

            ident = cst.tile([128, 128], bf16, name="ident")
            make_identity(nc, ident[:])
            ones1 = cst.tile([1, 128], bf16, name="ones1")
            nc.vector.memset(ones1[:], 1.0)
            ones128 = cst.tile([128, 1], bf16, name="ones128")
            nc.vector.memset(ones128[:], 1.0)
            epsc = cst.tile([128, 1], f32, name="epsc")
            nc.vector.memset(epsc[:], 1e-5)
            oi512 = cst.tile([128, 1], bf16, name="oi512")
            nc.vector.memset(oi512[:], 1.0 / 512)
            oi1024 = cst.tile([128, 1], bf16, name="oi1024")
            nc.vector.memset(oi1024[:], 1.0 / 1024)

            # ---------- weight stream (issue early) ----------
            xeT = ac.tile([128, 8, OWN], bf16, tag="xe", bufs=1, name="xeT")
            nc.sync.dma_start(out=xeT[:],
                              in_=x_t.rearrange("(c p) t -> p c t", c=8))
            ewT = wp.tile([128, 8, D], bf16, tag="wpk", bufs=2, name="ewT")
            nc.sync.dma_start(out=ewT[:],
                              in_=ew_t.rearrange("(c p) f -> p c f", c=8))

            def load_cols(dram_ap, n, tag, rows=128):
                ts = []
                for c in range(n):
                    t = wp.tile([rows, 1], f32, tag=f"{tag}{c}", bufs=1,
                                name=f"{tag}{c}")
                    nc.sync.dma_start(out=t[:],
                                      in_=dram_ap[rows * c:rows * (c + 1), :])
                    ts.append(t)
                return ts

            ebs = load_cols(ebias_t, 4, "ebias")
            eg2s = load_cols(eg2, 4, "eg2")
            eb2s = load_cols(eb2, 4, "eb2")

            wpk = [None] * DEPTH
            # weight DMA pieces, in consumption order
            PIECES = [(OKV, OMV), (OMV, OW1), (OW1, OWOH), (OWOH, FTOT)]

            def load_layer_weights(l):
                t = wp.tile([128, FTOT], bf16, tag="wpk", bufs=2,
                            name=f"wpk{l}")
                for (a, b) in PIECES:
                    nc.sync.dma_start(out=t[:, a:b], in_=wpack_t[l, :, a:b])
                return t

            wpk[0] = load_layer_weights(0)

            # ---------- helpers ----------
            def rstd_row(var_view, rstd_view):
                """rstd = exp(-0.5 * ln(var + eps)) on ACT."""
                nc.scalar.activation(out=rstd_view, in_=var_view, func=AF.Ln,
                                     bias=epsc[0:1, :])
                nc.scalar.activation(out=rstd_view, in_=rstd_view, func=AF.Exp,
                                     scale=-0.5)

            def bcast_row(row_bf16, T, nm):
                """[1,T] bf16 -> [128,T] bf16 via ones-matmul."""
                ps = pp.tile([128, T], f32, tag="g", name=f"bc_{nm}")
                nc.tensor.matmul(ps[:], ones1[:], row_bf16, start=True,
                                 stop=True)
                s = ac.tile([128, T], bf16, tag="bcb", bufs=3, name=f"bcs_{nm}")
                nc.vector.tensor_copy(out=s[:], in_=ps[:])
                return s

            def ln_scale(xs, T, oi, nm):
                """Scale-only feature-major LN over C chunks of [128, T].

                xs must be bf16 APs. Returns (y tiles, S_sbuf [1,T] f32).
                """
                C = len(xs)
                S = pp.tile([1, T], f32, tag="g", name=f"S_{nm}")
                for c in range(C):
                    nc.tensor.matmul(S[:], oi[:], xs[c], start=(c == 0),
                                     stop=(c == C - 1))
                Q = pp.tile([1, T], f32, tag="g", name=f"Q_{nm}")
                for c in range(C):
                    sq = ac.tile([128, T], bf16, tag="lnsq", bufs=2, name="sq")
                    nc.vector.tensor_mul(out=sq[:], in0=xs[c], in1=xs[c])
                    nc.tensor.matmul(Q[:], oi[:], sq[:], start=(c == 0),
                                     stop=(c == C - 1))
                st = ac.tile([1, 2 * T], f32, tag="st_ln", bufs=1,
                             name=f"st_{nm}")
                var, rstd = st[:, 0:T], st[:, T:2 * T]
                nc.vector.tensor_mul(out=var, in0=S[:], in1=S[:])
                nc.vector.tensor_sub(out=var, in0=Q[:], in1=var)
                rstd_row(var, rstd)
                rb = ac.tile([1, T], bf16, tag="rb_ln", bufs=2,
                             name=f"rb_{nm}")
                nc.vector.tensor_copy(out=rb[:], in_=rstd)
                rB = bcast_row(rb[:], T, nm)
                ys = []
                for c in range(C):
                    y = ac.tile([128, T], bf16, tag=f"y_ln{c}", bufs=2,
                                name=f"y_{nm}{c}")
                    nc.vector.tensor_mul(out=y[:], in0=xs[c], in1=rB[:])
                    ys.append(y)
                return ys, st

            _mark(nc, "embed")
            # ---------- embed ----------
            xe = [xeT[:, c, :] for c in range(8)]
            y1e, _ = ln_scale(xe, OWN, oi1024, "emb")
            ews = [ewT[:, c, :] for c in range(8)]
            # embed LN2: full affine feature-major LN (output is the
            # residual base, so the gain/bias cannot be folded away).
            tok = [ac.tile([128, TOK], bf16, tag=f"tok{c}", bufs=1,
                           name=f"tok{c}") for c in range(4)]
            xb2 = []
            for mc in range(4):
                ps = pp.tile([128, OWN], f32, tag="ff", bufs=6, name=f"embp{mc}")
                for kc in range(8):
                    nc.tensor.matmul(ps[:], ews[kc][:, 128 * mc:128 * (mc + 1)],
                                     y1e[kc][:], start=(kc == 0), stop=(kc == 7))
                xb = ac.tile([128, OWN], bf16, tag=f"y_ln{mc}", bufs=2,
                             name=f"exb{mc}")
                nc.vector.tensor_scalar_add(out=xb[:], in0=ps[:],
                                            scalar1=ebs[mc][:])
                xb2.append(xb)
            S = pp.tile([1, OWN], f32, tag="g", name="eS")
            for c in range(4):
                nc.tensor.matmul(S[:], oi512[:], xb2[c][:], start=(c == 0),
                                 stop=(c == 3))
            Q = pp.tile([1, OWN], f32, tag="g", name="eQ")
            for c in range(4):
                sq = ac.tile([128, OWN], bf16, tag="lnsq", bufs=2, name="esq")
                nc.vector.tensor_mul(out=sq[:], in0=xb2[c][:], in1=xb2[c][:])
                nc.tensor.matmul(Q[:], oi512[:], sq[:], start=(c == 0),
                                 stop=(c == 3))
            est = ac.tile([1, 2 * OWN], f32, tag="st_ln", bufs=1, name="est")
            evar, erstd = est[:, 0:OWN], est[:, OWN:2 * OWN]
            nc.vector.tensor_mul(out=evar, in0=S[:], in1=S[:])
            nc.vector.tensor_sub(out=evar, in0=Q[:], in1=evar)
            rstd_row(evar, erstd)
            epair = ac.tile([1, 2 * OWN], bf16, tag="epair", bufs=1,
                            name="epair")
            nc.vector.tensor_copy(out=epair[:, 0:OWN], in_=erstd)
            emrs = ac.tile([1, OWN], f32, tag="emrs", bufs=1, name="emrs")
            nc.vector.tensor_mul(out=emrs[:], in0=S[:], in1=erstd)
            nc.vector.tensor_copy(out=epair[:, OWN:2 * OWN], in_=emrs[:])
            erB = bcast_row(epair[:, 0:OWN], OWN, "er")
            emB = bcast_row(epair[:, OWN:2 * OWN], OWN, "em")
            for c in range(4):
                t1 = ac.tile([128, OWN], bf16, tag="lnsq", bufs=2, name="et1")
                nc.vector.tensor_mul(out=t1[:], in0=xb2[c][:], in1=erB[:])
                nc.vector.tensor_sub(out=t1[:], in0=t1[:], in1=emB[:])
                nc.vector.tensor_scalar(out=tok[c][:, 0:OWN], in0=t1[:],
                                        scalar1=eg2s[c][:], scalar2=eb2s[c][:],
                                        op0=OP.mult, op1=OP.add)
            for c in range(4):
                nc.sync.dma_start(out=tok[c][:, OWN:TOK],
                                  in_=fus_t[128 * c:128 * (c + 1), :])

            wpk[1] = load_layer_weights(1)

            # ---------- layers ----------
            psO_prev = None
            for l in range(DEPTH):
                w = wpk[l]
                wkv = [w[:, OKV + 1024 * c:OKV + 1024 * (c + 1)]
                       for c in range(4)]
                wq = [w[:, OQ + 512 * c:OQ + 512 * (c + 1)] for c in range(4)]
                mvo = [w[:, OMV + 512 * c:OMV + 512 * (c + 1)]
                       for c in range(4)]
                won = [w[:, OWN_ + 512 * c:OWN_ + 512 * (c + 1)]
                       for c in range(4)]

                def w1x(j, kc):
                    a = OW1 + 1024 * j + 256 * kc
                    return w[:, a:a + 128]

                def w1g(j, kc):
                    a = OW1 + 1024 * j + 256 * kc + 128
                    return w[:, a:a + 128]

                woh = [w[0:64, OWOH + 512 * h:OWOH + 512 * (h + 1)]
                       for h in range(H)]
                w2 = [w[:, OW2 + 512 * j:OW2 + 512 * (j + 1)]
                      for j in range(W2T)]

                _mark(nc, f"L{l}:residual+ln1")
                # residual from previous layer's FF2
                if psO_prev is not None:
                    for c in range(4):
                        nc.vector.tensor_add(out=tok[c][:], in0=tok[c][:],
                                             in1=psO_prev[c][:])
                    psO_prev = None

                # LN1 (scale-only)
                y1, _ = ln_scale([tok[c][:] for c in range(4)], TOK, oi512,
                                 f"l1_{l}")

                # ysum (row-sums of y1 own cols) -> EX1
                P1 = ac.tile([128, 4], f32, tag="P1", bufs=2, name="P1")
                yscr = ac.tile([128, OWN], bf16, tag="yscr", bufs=1,
                               name="yscr")
                for c in range(4):
                    nc.scalar.activation(out=yscr[:], in_=y1[c][:, 0:OWN],
                                         func=AF.Copy,
                                         accum_out=P1[:, c:c + 1])
                pin1 = dramp.tile([128, 4], f32, tag="pin1", bufs=2,
                                  name="pin1")
                nc.sync.dma_start(out=pin1[:], in_=P1[:])
                R1a = ac.tile([128, 4, 4], f32, tag="R1a", bufs=2, name="R1a")
                if use_cc:
                    pout1 = dramp.tile([4 * 128, 4], f32, tag="pout1", bufs=2,
                                       name="pout1")
                    nc.gpsimd.collective_compute(
                        "AllGather", OP.bypass, replica_groups=rg,
                        ins=[pin1.opt()], outs=[pout1.opt()])
                    nc.sync.dma_start(
                        out=R1a[:], in_=pout1.rearrange("(r p) f -> p r f", r=4))
                else:
                    nc.sync.dma_start(
                        out=R1a[:],
                        in_=pin1.rearrange("(r p) f -> p r f", r=1)
                        .to_broadcast((128, 4, 4)))

                _mark(nc, f"L{l}:attn")
                # attention matmuls
                kt = []
                for mc in range(4):
                    ps = pp.tile([128, OWN], f32, tag="g", name=f"kt{mc}")
                    for kc in range(4):
                        nc.tensor.matmul(ps[:],
                                         wkv[kc][:, 128 * mc:128 * (mc + 1)],
                                         y1[kc][:, 0:OWN],
                                         start=(kc == 0), stop=(kc == 3))
                    s = ac.tile([128, OWN], bf16, tag=f"kt{mc}", bufs=1,
                                name=f"ktb{mc}")
                    nc.vector.tensor_copy(out=s[:], in_=ps[:])
                    kt.append(s)
                qf = []
                for mc in range(4):
                    ps = pp.tile([128, FUS], f32, tag="g", name=f"qf{mc}")
                    for kc in range(4):
                        nc.tensor.matmul(ps[:],
                                         wq[kc][:, 128 * mc:128 * (mc + 1)],
                                         y1[kc][:, OWN:TOK],
                                         start=(kc == 0), stop=(kc == 3))
                    s = ac.tile([128, 32], bf16, tag=f"qf{mc}", bufs=1,
                                name=f"qfb{mc}")
                    nc.vector.memset(s[:, FUS:32], 0.0)
                    nc.vector.tensor_copy(out=s[:, 0:FUS], in_=ps[:])
                    qf.append(s)
                V = []
                for i, (a, b) in enumerate(tok_chunks):
                    m = b - a
                    ps = pp.tile([128, D], f32, tag="g", name=f"v{i}")
                    for kc in range(4):
                        nc.tensor.matmul(ps[0:m, :], y1[kc][:, a:b],
                                         wkv[kc][:, D:2 * D],
                                         start=(kc == 0), stop=(kc == 3))
                    s = ac.tile([128, D], bf16, tag=f"V{i}", bufs=1,
                                name=f"Vb{i}")
                    nc.vector.tensor_copy(out=s[0:m, :], in_=ps[0:m, :])
                    V.append(s)
                # scores + exp
                E, lacc = [], []
                for t in range(2):
                    sp = pp.tile([128, OWN], f32, tag="g", name=f"sp{t}")
                    for i in range(4):
                        h = 4 * t + i
                        ch, base = h // 2, (h % 2) * 64
                        nc.tensor.matmul(sp[32 * i:32 * i + 32, :],
                                         qf[ch][base:base + 64, 0:32],
                                         kt[ch][base:base + 64, :],
                                         start=True, stop=True,
                                         tile_position=(base, 32 * i))
                    e = ac.tile([128, OWN], bf16, tag=f"e{t}", bufs=1,
                                name=f"e{t}")
                    la = ac.tile([128, 1], f32, tag=f"la{t}", bufs=2,
                                 name=f"la{t}")
                    nc.scalar.activation(out=e[:], in_=sp[:], func=AF.Exp,
                                         accum_out=la[:])
                    E.append(e)
                    lacc.append(la)
                ET = [[None] * 3 for _ in range(2)]
                for t in range(2):
                    for j in range(3):
                        pt = pp.tile([128, 128], bf16, tag="g", name=f"et{t}{j}")
                        nc.tensor.transpose(pt[:], E[t][:, 128 * j:128 * (j + 1)],
                                            ident[:])
                        s = ac.tile([128, 128], bf16, tag=f"ET{t}{j}", bufs=1,
                                    name=f"ETb{t}{j}")
                        nc.vector.tensor_copy(out=s[:], in_=pt[:])
                        ET[t][j] = s
                # P2 = [l0, l1, ACC0, ACC1]
                P2 = ac.tile([128, 130], f32, tag="P2", bufs=2, name="P2")
                nc.vector.tensor_copy(out=P2[:, 0:1], in_=lacc[0][:])
                nc.vector.tensor_copy(out=P2[:, 1:2], in_=lacc[1][:])
                for t in range(2):
                    acc = pp.tile([128, 64], f32, tag="g", name=f"acc{t}")
                    for i in range(4):
                        h = 4 * t + i
                        for j in range(3):
                            nc.tensor.matmul(acc[32 * i:32 * i + 32, :],
                                             ET[t][j][:, 32 * i:32 * i + 32],
                                             V[j][:, DH * h:DH * (h + 1)],
                                             start=(j == 0), stop=(j == 2),
                                             tile_position=(0, 32 * i))
                    nc.vector.tensor_copy(out=P2[:, 2 + 64 * t:66 + 64 * t],
                                          in_=acc[:])
                pin2 = dramp.tile([128, 130], f32, tag="pin2", bufs=2,
                                  name="pin2")
                nc.sync.dma_start(out=pin2[:], in_=P2[:])
                R2a = ac.tile([128, 4, 130], f32, tag="R2a", bufs=2,
                              name="R2a")
                if use_cc:
                    pout2 = dramp.tile([4 * 128, 130], f32, tag="pout2",
                                       bufs=2, name="pout2")
                    nc.gpsimd.collective_compute(
                        "AllGather", OP.bypass, replica_groups=rg,
                        ins=[pin2.opt()], outs=[pout2.opt()])
                    nc.sync.dma_start(
                        out=R2a[:], in_=pout2.rearrange("(r p) f -> p r f", r=4))
                else:
                    nc.sync.dma_start(
                        out=R2a[:],
                        in_=pin2.rearrange("(r p) f -> p r f", r=1)
                        .to_broadcast((128, 4, 130)))

                _mark(nc, f"L{l}:vfu+du")
                # vfu: fusion-key V column sums (local)
                vfub = ac.tile([128, 4], bf16, tag="vfub", bufs=2, name="vfub")
                for c in range(4):
                    ps = pp.tile([128, 1], f32, tag="g", name=f"vf{c}")
                    nc.tensor.matmul(ps[:], V[3][0:FUS, 128 * c:128 * (c + 1)],
                                     ones128[0:FUS, :], start=True, stop=True)
                    nc.vector.tensor_copy(out=vfub[:, c:c + 1], in_=ps[:])

                # EX1 return -> ysum_all -> du
                ysp = ac.tile([128, 2, 4], f32, tag="ysp", bufs=2, name="ysp")
                nc.vector.tensor_add(out=ysp[:], in0=R1a[:, 0:2, :],
                                     in1=R1a[:, 2:4, :])
                ysb = ac.tile([128, 4], bf16, tag="ysb", bufs=2, name="ysb")
                nc.vector.tensor_add(out=ysb[:], in0=ysp[:, 0, :],
                                     in1=ysp[:, 1, :])
                duP = pp.tile([128, 4], f32, tag="g", name="duP")
                for mc in range(4):
                    for kc in range(4):
                        nc.tensor.matmul(duP[:, mc:mc + 1],
                                         mvo[kc][:, 128 * mc:128 * (mc + 1)],
                                         ysb[:, kc:kc + 1],
                                         start=(kc == 0), stop=False)
                    for kc in range(4):
                        nc.tensor.matmul(duP[:, mc:mc + 1],
                                         won[kc][:, 128 * mc:128 * (mc + 1)],
                                         vfub[:, kc:kc + 1],
                                         start=False, stop=(kc == 3))
                dub = ac.tile([128, 4], f32, tag="dub", bufs=2, name="dub")
                nc.vector.tensor_copy(out=dub[:], in_=duP[:])
                dus = [dub[:, c:c + 1] for c in range(4)]

                # x2 own = tok + du; LN2-own stats
                for c in range(4):
                    nc.vector.tensor_scalar_add(out=tok[c][:, 0:OWN],
                                                in0=tok[c][:, 0:OWN],
                                                scalar1=dus[c])
                S2 = pp.tile([1, OWN], f32, tag="g", name="S2")
                for c in range(4):
                    nc.tensor.matmul(S2[:], oi512[:], tok[c][:, 0:OWN],
                                     start=(c == 0), stop=(c == 3))
                Q2 = pp.tile([1, OWN], f32, tag="g", name="Q2")
                for c in range(4):
                    sq = ac.tile([128, OWN], bf16, tag="lnsq", bufs=2,
                                 name="sq2")
                    nc.vector.tensor_mul(out=sq[:], in0=tok[c][:, 0:OWN],
                                         in1=tok[c][:, 0:OWN])
                    nc.tensor.matmul(Q2[:], oi512[:], sq[:],
                                     start=(c == 0), stop=(c == 3))
                st2 = ac.tile([1, 2 * TOK], f32, tag="st2", bufs=1, name="st2")
                var2, rstd2 = st2[:, 0:TOK], st2[:, TOK:2 * TOK]
                nc.vector.tensor_mul(out=var2[:, 0:OWN], in0=S2[:],
                                     in1=S2[:])
                nc.vector.tensor_sub(out=var2[:, 0:OWN], in0=Q2[:],
                                     in1=var2[:, 0:OWN])
                rstd_row(var2[:, 0:OWN], rstd2[:, 0:OWN])
                rb2 = ac.tile([1, TOK], bf16, tag="rb2", bufs=1, name="rb2")
                nc.vector.tensor_copy(out=rb2[:, 0:OWN], in_=rstd2[:, 0:OWN])
                rB2o = bcast_row(rb2[:, 0:OWN], OWN, f"r2o_{l}")
                y2 = []
                for c in range(4):
                    y = ac.tile([128, OWN], bf16, tag=f"y2{c}", bufs=1,
                                name=f"y2_{c}")
                    nc.vector.tensor_mul(out=y[:], in0=tok[c][:, 0:OWN],
                                         in1=rB2o[:])
                    y2.append(y)

                _mark(nc, f"L{l}:ff1own")
                # FF1-own matmul stream
                pxs, pgs = [], []
                for j in range(W2T):
                    px = pp.tile([128, OWN], f32, tag="ff", bufs=6, name=f"px{j}")
                    pg = pp.tile([128, OWN], f32, tag="ff", bufs=6, name=f"pg{j}")
                    for kc in range(4):
                        nc.tensor.matmul(px[:], w1x(j, kc), y2[kc][:],
                                         start=(kc == 0), stop=(kc == 3))
                    for kc in range(4):
                        nc.tensor.matmul(pg[:], w1g(j, kc), y2[kc][:],
                                         start=(kc == 0), stop=(kc == 3))
                    pxs.append(px)
                    pgs.append(pg)

                _mark(nc, f"L{l}:gelu_own")
                # gelu + gate for own cols (ACT switches to gelu table here)
                gt = []
                for j in range(W2T):
                    gg = ac.tile([128, OWN], bf16, tag="gg", bufs=3,
                                 name=f"gg{j}")
                    nc.scalar.activation(out=gg[:], in_=pgs[j][:],
                                         func=AF.Gelu)
                    g = ac.tile([128, TOK], bf16, tag="gt", bufs=12,
                                name=f"gt{j}")
                    nc.vector.tensor_mul(out=g[:, 0:OWN], in0=gg[:],
                                         in1=pxs[j][:])
                    gt.append(g)

                _mark(nc, f"L{l}:ex2ret")
                # EX2 return -> fusion attention delta
                T01 = ac.tile([128, 2, 130], f32, tag="cmb", bufs=2,
                              name="T01")
                nc.vector.tensor_add(out=T01[:], in0=R2a[:, 0:2, :],
                                     in1=R2a[:, 2:4, :])
                PT = ac.tile([128, 130], f32, tag="PT", bufs=2, name="PT")
                nc.vector.tensor_add(out=PT[:], in0=T01[:, 0, :],
                                     in1=T01[:, 1, :])
                linv = ac.tile([128, 2], f32, tag="linv", bufs=2, name="linv")
                nc.vector.reciprocal(out=linv[:], in_=PT[:, 0:2])
                ofT = []
                for t in range(2):
                    of = ac.tile([128, 64], bf16, tag=f"of{t}", bufs=1,
                                 name=f"of{t}")
                    nc.vector.tensor_scalar_mul(out=of[:],
                                                in0=PT[:, 2 + 64 * t:66 + 64 * t],
                                                scalar1=linv[:, t:t + 1])
                    pt = pp.tile([64, 128], bf16, tag="g", name=f"oft{t}")
                    nc.tensor.transpose(pt[:], of[:], ident[:])
                    s = ac.tile([64, 128], bf16, tag=f"ofT{t}", bufs=1,
                                name=f"ofTb{t}")
                    nc.vector.tensor_copy(out=s[:], in_=pt[:])
                    ofT.append(s)
                df = pp.tile([FUS, D], f32, tag="g", name="df")
                for h in range(H):
                    t, i = h // 4, h % 4
                    nc.tensor.matmul(df[:], ofT[t][:, 32 * i:32 * i + FUS],
                                     woh[h], start=(h == 0), stop=(h == 7))
                dfb = ac.tile([FUS, D], bf16, tag="dfb", bufs=2, name="dfb")
                nc.vector.tensor_copy(out=dfb[:], in_=df[:])
                dftps = []
                for c in range(4):
                    pt = pp.tile([128, FUS], bf16, tag="g", name=f"dft{c}")
                    nc.tensor.transpose(pt[:], dfb[0:FUS, 128 * c:128 * (c + 1)],
                                        ident[0:FUS, 0:FUS])
                    s = ac.tile([128, FUS], bf16, tag=f"dftp{c}", bufs=1,
                                name=f"dftps{c}")
                    nc.vector.tensor_copy(out=s[:], in_=pt[:])
                    dftps.append(s)
                for c in range(4):
                    nc.vector.tensor_add(out=tok[c][:, OWN:TOK],
                                         in0=tok[c][:, OWN:TOK],
                                         in1=dftps[c][:])

                _mark(nc, f"L{l}:fus_stats")
                # LN2-fus stats + rstd (ln/exp table) after all own gelus
                Sf = pp.tile([1, FUS], f32, tag="g", name="Sf")
                for c in range(4):
                    nc.tensor.matmul(Sf[:], oi512[:], tok[c][:, OWN:TOK],
                                     start=(c == 0), stop=(c == 3))
                Qf = pp.tile([1, FUS], f32, tag="g", name="Qf")
                for c in range(4):
                    sq = ac.tile([128, FUS], bf16, tag="sqf", bufs=2,
                                 name="sqf")
                    nc.vector.tensor_mul(out=sq[:], in0=tok[c][:, OWN:TOK],
                                         in1=tok[c][:, OWN:TOK])
                    nc.tensor.matmul(Qf[:], oi512[:], sq[:], start=(c == 0),
                                     stop=(c == 3))
                nc.vector.tensor_mul(out=var2[:, OWN:TOK], in0=Sf[:],
                                     in1=Sf[:])
                nc.vector.tensor_sub(out=var2[:, OWN:TOK], in0=Qf[:],
                                     in1=var2[:, OWN:TOK])
                rstd_row(var2[:, OWN:TOK], rstd2[:, OWN:TOK])
                nc.vector.tensor_copy(out=rb2[:, OWN:TOK],
                                      in_=rstd2[:, OWN:TOK])
                rB2f = bcast_row(rb2[:, OWN:TOK], FUS, f"r2f_{l}")

                # FF1-fus (unscaled input, post-scale by rB2f)
                for j in range(W2T):
                    pxf = pp.tile([128, FUS], f32, tag="g", name=f"pxf{j}")
                    for kc in range(4):
                        nc.tensor.matmul(pxf[:], w1x(j, kc),
                                         tok[kc][:, OWN:TOK],
                                         start=(kc == 0), stop=(kc == 3))
                    pgf = pp.tile([128, FUS], f32, tag="g", name=f"pgf{j}")
                    for kc in range(4):
                        nc.tensor.matmul(pgf[:], w1g(j, kc),
                                         tok[kc][:, OWN:TOK],
                                         start=(kc == 0), stop=(kc == 3))
                    xfs = ac.tile([128, FUS], bf16, tag="xfs", bufs=2,
                                  name=f"xfs{j}")
                    nc.vector.tensor_mul(out=xfs[:], in0=pxf[:], in1=rB2f[:])
                    gfs = ac.tile([128, FUS], bf16, tag="gfs", bufs=2,
                                  name=f"gfs{j}")
                    nc.vector.tensor_mul(out=gfs[:], in0=pgf[:], in1=rB2f[:])
                    ggf = ac.tile([128, FUS], bf16, tag="ggf", bufs=2,
                                  name=f"ggf{j}")
                    nc.scalar.activation(out=ggf[:], in_=gfs[:], func=AF.Gelu)
                    nc.vector.tensor_mul(out=gt[j][:, OWN:TOK], in0=ggf[:],
                                         in1=xfs[:])

                _mark(nc, f"L{l}:ff2")
                # FF2 (own + fus cols together now that gt complete)
                psO_prev = []
                for c in range(4):
                    psO = pp.tile([128, TOK], f32, tag="ff", bufs=6, name=f"fo{c}")
                    for j in range(W2T):
                        nc.tensor.matmul(psO[:], w2[j][:, 128 * c:128 * (c + 1)],
                                         gt[j][:], start=(j == 0),
                                         stop=(j == W2T - 1))
                    psO_prev.append(psO)

                # prefetch next layer weights (slot is free by now, so the
                # dma issue does not block the SP queue)
                if l + 1 < DEPTH:
                    wpk[l + 1] = load_layer_weights(l + 1)

            _mark(nc, "pool")
            # ---------- pool ----------
            pw = wp.tile([128, 12288], bf16, tag="wpk", bufs=2, name="pw")
            nc.sync.dma_start(out=pw[:], in_=ppack_t)
            pkv = [pw[:, POKV + 1024 * c:POKV + 1024 * (c + 1)]
                   for c in range(4)]
            pwoh = [pw[0:64, POWOH + 512 * h:POWOH + 512 * (h + 1)]
                    for h in range(H)]
            pwon = [pw[:, POWON + 512 * c:POWON + 512 * (c + 1)]
                    for c in range(4)]
            pmvo = [pw[:, POMV + 512 * c:POMV + 512 * (c + 1)]
                    for c in range(4)]
            pq2s = load_cols(pq2_t, 4, "pq2")

            for c in range(4):
                nc.vector.tensor_add(out=tok[c][:], in0=tok[c][:],
                                     in1=psO_prev[c][:])
            tokn, _ = ln_scale([tok[c][:] for c in range(4)], TOK, oi512, "fin")

            # ysum exchange
            Pp = ac.tile([128, 4], f32, tag="P1", bufs=2, name="Pp")
            pscr = ac.tile([128, OWN], bf16, tag="yscr", bufs=1, name="pscr")
            for c in range(4):
                nc.scalar.activation(out=pscr[:], in_=tokn[c][:, 0:OWN],
                                     func=AF.Copy, accum_out=Pp[:, c:c + 1])
            pinp = dramp.tile([128, 4], f32, tag="pin1", bufs=2, name="pinp")
            nc.sync.dma_start(out=pinp[:], in_=Pp[:])
            Rpa = ac.tile([128, 4, 4], f32, tag="R1a", bufs=2, name="Rpa")
            if use_cc:
                poutp = dramp.tile([4 * 128, 4], f32, tag="pout1", bufs=2,
                                   name="poutp")
                nc.gpsimd.collective_compute(
                    "AllGather", OP.bypass, replica_groups=rg,
                    ins=[pinp.opt()], outs=[poutp.opt()])
                nc.sync.dma_start(
                    out=Rpa[:], in_=poutp.rearrange("(r p) f -> p r f", r=4))
            else:
                nc.sync.dma_start(
                    out=Rpa[:],
                    in_=pinp.rearrange("(r p) f -> p r f", r=1)
                    .to_broadcast((128, 4, 4)))

            # fusion V for pool (only fusion rows needed)
            psv = pp.tile([128, D], f32, tag="g", name="psv")
            for kc in range(4):
                nc.tensor.matmul(psv[0:FUS, :], tokn[kc][:, OWN:TOK],
                                 pkv[kc][:, D:2 * D],
                                 start=(kc == 0), stop=(kc == 3))
            Vpf = ac.tile([FUS, D], bf16, tag="Vpf", bufs=1, name="Vpf")
            nc.vector.tensor_copy(out=Vpf[:], in_=psv[0:FUS, :])
            pvfub = ac.tile([128, 4], bf16, tag="vfub", bufs=2, name="pvfub")
            for c in range(4):
                ps = pp.tile([128, 1], f32, tag="g", name=f"pvf{c}")
                nc.tensor.matmul(ps[:], Vpf[0:FUS, 128 * c:128 * (c + 1)],
                                 ones128[0:FUS, :], start=True, stop=True)
                nc.vector.tensor_copy(out=pvfub[:, c:c + 1], in_=ps[:])

            pysp = ac.tile([128, 2, 4], f32, tag="ysp", bufs=2, name="pysp")
            nc.vector.tensor_add(out=pysp[:], in0=Rpa[:, 0:2, :],
                                 in1=Rpa[:, 2:4, :])
            pysb = ac.tile([128, 4], bf16, tag="ysb", bufs=2, name="pysb")
            nc.vector.tensor_add(out=pysb[:], in0=pysp[:, 0, :],
                                 in1=pysp[:, 1, :])
            for mc in range(4):
                ps = pp.tile([128, 1], f32, tag="g", name=f"pdu{mc}")
                for kc in range(4):
                    nc.tensor.matmul(ps[:],
                                     pmvo[kc][:, 128 * mc:128 * (mc + 1)],
                                     pysb[:, kc:kc + 1],
                                     start=(kc == 0), stop=False)
                for kc in range(4):
                    nc.tensor.matmul(ps[:],
                                     pwon[kc][:, 128 * mc:128 * (mc + 1)],
                                     pvfub[:, kc:kc + 1],
                                     start=False, stop=(kc == 3))
                s = ac.tile([128, 1], f32, tag=f"du{mc}", bufs=2,
                            name=f"pdub{mc}")
                nc.vector.tensor_copy(out=s[:], in_=ps[:])
                nc.sync.dma_start(out=out_u[128 * mc:128 * (mc + 1), :],
                                  in_=s[:])

            # fusion-key attention for return token 2 (all local)
            kf = []
            for mc in range(4):
                ps = pp.tile([128, FUS], f32, tag="g", name=f"pkf{mc}")
                for kc in range(4):
                    nc.tensor.matmul(ps[:], pkv[kc][:, 128 * mc:128 * (mc + 1)],
                                     tokn[kc][:, OWN:TOK],
                                     start=(kc == 0), stop=(kc == 3))
                s = ac.tile([128, FUS], bf16, tag=f"kf{mc}", bufs=1,
                            name=f"kfb{mc}")
                nc.vector.tensor_copy(out=s[:], in_=ps[:])
                kf.append(s)
            q2 = []
            for mc in range(4):
                s = ac.tile([128, 32], bf16, tag=f"qf{mc}", bufs=1,
                            name=f"q2b{mc}")
                nc.vector.memset(s[:, 1:32], 0.0)
                nc.vector.tensor_copy(out=s[:, 0:1], in_=pq2s[mc][:])
                q2.append(s)
            e2, l2 = [], []
            for t in range(2):
                sp = pp.tile([128, FUS], f32, tag="g", name=f"ps2{t}")
                for i in range(4):
                    h = 4 * t + i
                    ch, base = h // 2, (h % 2) * 64
                    nc.tensor.matmul(sp[32 * i:32 * i + 32, :],
                                     q2[ch][base:base + 64, 0:32],
                                     kf[ch][base:base + 64, :],
                                     start=True, stop=True,
                                     tile_position=(base, 32 * i))
                e = ac.tile([128, FUS], bf16, tag=f"e2{t}", bufs=1,
                            name=f"e2{t}")
                la = ac.tile([128, 1], f32, tag=f"la{t}", bufs=2,
                             name=f"pla{t}")
                nc.scalar.activation(out=e[:], in_=sp[:], func=AF.Exp,
                                     accum_out=la[:])
                e2.append(e)
                l2.append(la)
            e2T = []
            for t in range(2):
                pt = pp.tile([FUS, 128], bf16, tag="g", name=f"pet{t}")
                nc.tensor.transpose(pt[:], e2[t][:], ident[:])
                s = ac.tile([FUS, 128], bf16, tag=f"e2T{t}", bufs=1,
                            name=f"e2Tb{t}")
                nc.vector.tensor_copy(out=s[:], in_=pt[:])
                e2T.append(s)
            ofT2 = []
            for t in range(2):
                acc = pp.tile([128, 64], f32, tag="g", name=f"pacc2{t}")
                for i in range(4):
                    h = 4 * t + i
                    nc.tensor.matmul(acc[32 * i:32 * i + 32, :],
                                     e2T[t][:, 32 * i:32 * i + 32],
                                     Vpf[0:FUS, DH * h:DH * (h + 1)],
                                     start=True, stop=True,
                                     tile_position=(0, 32 * i))
                li = ac.tile([128, 1], f32, tag="linv2", bufs=2, name=f"pli{t}")
                nc.vector.reciprocal(out=li[:], in_=l2[t][:])
                of = ac.tile([128, 64], bf16, tag=f"pof{t}", bufs=1,
                             name=f"pof{t}")
                nc.vector.tensor_scalar_mul(out=of[:], in0=acc[:],
                                            scalar1=li[:])
                pt = pp.tile([64, 128], bf16, tag="g", name=f"poft{t}")
                nc.tensor.transpose(pt[:], of[:], ident[:])
                s = ac.tile([64, 128], bf16, tag=f"pofT{t}", bufs=1,
                            name=f"pofTb{t}")
                nc.vector.tensor_copy(out=s[:], in_=pt[:])
                ofT2.append(s)
            P2p = pp.tile([1, D], f32, tag="g", name="P2p")
            for h in range(H):
                t, i = h // 4, h % 4
                nc.tensor.matmul(P2p[:], ofT2[t][:, 32 * i:32 * i + 1],
                                 pwoh[h], start=(h == 0), stop=(h == 7))
            p2s = ac.tile([1, D], f32, tag="p2s", bufs=1, name="p2s")
            nc.vector.tensor_copy(out=p2s[:], in_=P2p[:])
            nc.sync.dma_start(out=out_f[:], in_=p2s[:])

    nc.compile()
    _built[key] = nc
    return nc


def _center(w):
    return w - w.mean(axis=0, keepdims=True)


def _prep_inputs(inputs):
    """Host-side prep: fold LN gains + centering into weights, pack."""
    I = {k: np.asarray(v) for k, v in inputs.items()}
    f32, f64 = np.float32, np.float64

    def bf(x):
        return np.ascontiguousarray(x).astype(BF)

    def col(x):
        return np.ascontiguousarray(np.asarray(x, f32).reshape(-1, 1))

    def cm(x, k, m):
        """[k*128, m] -> [128, k*m] chunk-major pack."""
        return x.reshape(k, 128, m).transpose(1, 0, 2).reshape(128, k * m)

    scale = DH ** -0.5
    packs = []
    for l in range(DEPTH):
        g = I["layers_attn_g"][l].astype(f64)
        wq = _center(I["layers_wq"][l].astype(f64) * g[:, None]) * scale
        wkv = _center(I["layers_wkv"][l].astype(f64) * g[:, None])
        wo = I["layers_wo"][l].astype(f64)
        won = wo / NALL
        mvo = wkv[:, D:] @ won
        fg = I["layers_ff_g"][l].astype(f64)
        w1 = _center(I["layers_ff_w1"][l].astype(f64) * fg[:, None])
        w2 = I["layers_ff_w2"][l].astype(f64)

        # w1 j-major pack: [j][kc][x 128 | g 128]
        w1jm = np.zeros((128, W2T * 1024), f64)
        x1 = np.zeros((512, IFFP), f64)
        gt_ = np.zeros((512, IFFP), f64)
        x1[:, :IFF] = w1[:, :IFF]
        gt_[:, :IFF] = w1[:, IFF:]
        for j in range(W2T):
            for kc in range(4):
                a = 1024 * j + 256 * kc
                w1jm[:, a:a + 128] = x1[128 * kc:128 * (kc + 1),
                                        128 * j:128 * (j + 1)]
                w1jm[:, a + 128:a + 256] = gt_[128 * kc:128 * (kc + 1),
                                               128 * j:128 * (j + 1)]
        woh = np.zeros((128, H * D), f64)
        woh[0:64, :] = wo.reshape(H, DH, D).transpose(1, 0, 2).reshape(64, H * D)
        w2p = np.pad(w2, ((0, IFFP - IFF), (0, 0)))
        pack = np.concatenate([
            cm(wkv, 4, 1024), cm(wq, 4, 512), cm(mvo, 4, 512),
            cm(won, 4, 512), w1jm, woh, cm(w2p, W2T, 512)], axis=1)
        packs.append(pack)
    wpack = bf(np.stack(packs))
    assert wpack.shape == (DEPTH, 128, FTOT), wpack.shape

    fgl = I["final_g"].astype(f64)
    pkv = _center(I["pool_wkv"].astype(f64) * fgl[:, None])
    pwo = I["pool_wo"].astype(f64)
    pwon = pwo / NALL
    pmvo = pkv[:, D:] @ pwon
    pwoh = np.zeros((128, H * D), f64)
    pwoh[0:64, :] = pwo.reshape(H, DH, D).transpose(1, 0, 2).reshape(64, H * D)
    ppack = bf(np.concatenate([
        cm(pkv, 4, 1024), pwoh, cm(pwon, 4, 512), cm(pmvo, 4, 512)], axis=1))
    assert ppack.shape == (128, 10240), ppack.shape

    # host-side pool query for return token 2 (row 2 = FUSION)
    ret = I["return_tokens"].astype(f32)
    g = I["pool_g"].astype(f32)
    mu = ret.mean(-1, keepdims=True)
    var = ((ret - mu) ** 2).mean(-1, keepdims=True)
    retn = (ret - mu) / np.sqrt(var + 1e-5) * g
    q2 = (retn[2] @ I["pool_wq"].astype(f32)) * scale

    shared = {
        "fus_t": bf(I["fusion_tokens"].astype(f64).T),
        "wpack": wpack,
        "ppack": ppack,
        "pool_q2": col(q2),
    }

    in_maps = []
    for c in range(N_CORES):
        b, q = c // 4, c % 4
        mod = "rna" if q < 2 else "atac"
        x = I[mod][b, (q % 2) * OWN:(q % 2 + 1) * OWN, :]  # [384, 1024]
        m = dict(shared)
        m["x_t"] = bf(x.astype(f64).T)
        ewf = _center(I[f"{mod}_w"].astype(f64)
                      * I[f"{mod}_ln1_g"].astype(f64)[:, None])
        m["emb_w"] = bf(ewf)
        m["emb_b"] = col(I[f"{mod}_b"].astype(f64)
                         + I[f"{mod}_ln1_b"].astype(f64)
                         @ I[f"{mod}_w"].astype(f64))
        m["eln2_g"] = col(I[f"{mod}_ln2_g"])
        m["eln2_b"] = col(I[f"{mod}_ln2_b"])
        in_maps.append(m)
    return in_maps, ret


def kernel(**inputs):
    from concourse import bass_utils
    nc = build(num_devices=N_CORES, use_cc=True)
    in_maps, ret = _prep_inputs(inputs)
    res = bass_utils.run_bass_kernel_spmd(nc, in_maps,
                                          core_ids=list(range(N_CORES)))
    out = np.zeros((B, 3, D), np.float32)
    for b in range(2):
        r = res.results[4 * b]
        u = r["out_u"][:, 0]
        f = r["out_f"][0]
        out[b, 0] = u + ret[0]
        out[b, 1] = u + ret[1]
        out[b, 2] = f + ret[2]
    return out
